# revision 16
# baseline (speedup 1.0000x reference)
"""Trainium2 Bass kernel for additive (Bahdanau) attention.

Reference computation (per batch b):
    qp = queries @ Wq                    # (Tq, H)
    kp = keys @ Wk                       # (Tk, H)
    scores[q,k] = sum_h wv[h] * tanh(qp[q,h] + kp[k,h])
    attn = softmax(scores masked to k < valid_lens[b])
    out = attn @ values                  # (Tq, D)

Shapes: B=8, Tq=128, Tk=512, D=256, H=256 (fp32).

Strategy (v2 — valid-length-balanced key-axis sharding):

The dominant cost is the (b, q, k, h) feature tensor, but keys with
k >= valid_lens[b] never influence the output, so only
sum_b ceil(len_b/128) 128-key chunks of work exist. The host enumerates
those chunks (b, kc), pads to a uniform U = ceil(n/8) per core with dummy
(fully-masked) chunks, and ships per-chunk inputs; every core runs the
same static program over U chunks.

Per chunk, on-core (feature axis H on partitions, two halves of 128):
  - qp/kp projections on TensorE from pre-transposed qT/kT slices;
  - qp+kp broadcast-add via DVE/GPSIMD tensor_tensor with step-0
    access-pattern broadcasts (qp broadcast along k, kp along q), the key
    columns split 88:40 between VectorE and GpSimd so both stay under the
    tanh pace;
  - tanh on ScalarE over [128, 2*32*128] tiles -> bf16 features;
  - wv contraction on TensorE with the feature tile as the stationary
    operand (lhsT [128h, 128k]) and the wv half [128,1] as the moving
    operand, accumulating transposed score columns scT[k, q] in PSUM;
  - exp(scT + bias) where bias = mask - M (M = sum|wv| bounds all scores,
    so no row max is needed and cross-chunk partials combine by plain
    summation); k is the partition axis so the mask is a per-partition
    bias on the one exp instruction;
  - attn-partial @ [values | 1] on TensorE: a ones-column appended to V
    accumulates the softmax denominator as column 256.
The host sums the per-chunk [128, 257] partials per batch and divides by
column 256 to produce the normalized output.
"""

import math
import numpy as np
import ml_dtypes
from contextlib import ExitStack

import concourse.bass as bass
import concourse.tile as tile
from concourse import bacc, mybir
from concourse import bass_utils

B, Tq, Tk, D, H = 8, 128, 512, 256, 256
N_CORES = 8
KC = 128          # keys per chunk
QG = 32           # queries per tanh group
NGRP = Tq // QG   # 4 groups per chunk
# Key columns of each add on DVE vs GpSimd. GpSimd's software tensor_tensor
# loop costs ~74ns per qi-row regardless of length, so it can only carry a
# small share before falling behind the tanh pace.
KS_DVE = 108
F32 = mybir.dt.float32
BF16 = mybir.dt.bfloat16
NEG_BIG = -1.0e9


def _bcast(ap_slice, axis_idx, count):
    """Insert a step-0 (broadcast) dim into an AP at free-axis position."""
    ap = list(ap_slice.ap)
    ap.insert(axis_idx, [0, count])
    return bass.AP(tensor=ap_slice.tensor, offset=ap_slice.offset, ap=ap)


def _emit(nc, tc, ins, out_dram, U):
    with ExitStack() as ctx:
        const = ctx.enter_context(tc.tile_pool(name="const", bufs=1))
        chunk_in = ctx.enter_context(tc.tile_pool(name="chunk_in", bufs=2))
        proj_sb = ctx.enter_context(tc.tile_pool(name="proj_sb", bufs=2))
        ssum_pool = ctx.enter_context(tc.tile_pool(name="ssum", bufs=2))
        feat_pool = ctx.enter_context(tc.tile_pool(name="feat", bufs=2))
        pt_pool = ctx.enter_context(tc.tile_pool(name="pt", bufs=2))
        out_pool = ctx.enter_context(tc.tile_pool(name="outs", bufs=2))
        proj_ps = ctx.enter_context(tc.tile_pool(name="proj_ps", bufs=2, space="PSUM"))
        sc_ps_pool = ctx.enter_context(tc.tile_pool(name="sc_ps", bufs=2, space="PSUM"))
        av_ps_pool = ctx.enter_context(tc.tile_pool(name="av_ps", bufs=2, space="PSUM"))

        # Dummy activation with no data dependencies: pulls the ACT table
        # load (~1.3us, tanh+exp share one set) off the first tanh's
        # critical path — it runs concurrently with the input DMAs.
        warm_sb = const.tile([1, 1], F32)
        nc.vector.memset(warm_sb, 0.0)
        nc.scalar.activation(warm_sb, warm_sb, mybir.ActivationFunctionType.Tanh)

        # Two HWDGE issue queues (sync + scalar). Chunk-0 critical path runs
        # kT/wk -> kp projection -> kp_rep chain, so those loads go first.
        wk_sb = const.tile([128, 2, H], BF16)
        wq_sb = const.tile([128, 2, H], BF16)
        wv_sb = const.tile([128, 2], BF16)

        for u in range(U):
            # ---- chunk input loads ----
            kT_sb = chunk_in.tile([128, 2, KC], BF16, tag="kT")
            nc.sync.dma_start(
                out=kT_sb, in_=ins["kT_u"][u].rearrange("(c p) k -> p c k", p=128)
            )
            qT_sb = chunk_in.tile([128, 2, Tq], BF16, tag="qT")
            nc.scalar.dma_start(
                out=qT_sb, in_=ins["qT_u"][u].rearrange("(c p) q -> p c q", p=128)
            )
            if u == 0:
                nc.sync.dma_start(
                    out=wk_sb, in_=ins["wk"].rearrange("(c p) h -> p c h", p=128)
                )
                nc.scalar.dma_start(
                    out=wq_sb, in_=ins["wq"].rearrange("(c p) h -> p c h", p=128)
                )
                nc.scalar.dma_start(
                    out=wv_sb, in_=ins["wv"].rearrange("(c p) -> p c", p=128)
                )
            v_sb = chunk_in.tile([128, D + 1], BF16, tag="v")
            nc.sync.dma_start(out=v_sb, in_=ins["v_u"][u])
            mb_sb = chunk_in.tile([128, 1], F32, tag="mb")
            nc.sync.dma_start(out=mb_sb, in_=ins["mb_u"][u])

            # ---- projections (kp first: it feeds the longer copy chain) ----
            qp_ps = proj_ps.tile([128, 2, Tq], F32, tag="qp_ps")
            kp_ps = proj_ps.tile([128, 2, KC], F32, tag="kp_ps")
            for half in range(2):
                hs = slice(half * 128, (half + 1) * 128)
                for dc in range(2):
                    nc.tensor.matmul(
                        kp_ps[:, half, :],
                        wk_sb[:, dc, hs],
                        kT_sb[:, dc, :],
                        start=(dc == 0),
                        stop=(dc == 1),
                    )
            for half in range(2):
                hs = slice(half * 128, (half + 1) * 128)
                for dc in range(2):
                    nc.tensor.matmul(
                        qp_ps[:, half, :],
                        wq_sb[:, dc, hs],
                        qT_sb[:, dc, :],
                        start=(dc == 0),
                        stop=(dc == 1),
                    )
            # On chunk 0 the scalar engine is still idle, so routing the
            # qpT PSUM->SBUF copy there lets the DVE kp_rep chain below run
            # concurrently and shortens the preamble.
            qpT_sb = proj_sb.tile([128, 2, Tq], BF16, tag="qpT")
            if u == 0:
                nc.scalar.copy(qpT_sb, qp_ps)
            else:
                nc.vector.tensor_copy(qpT_sb, qp_ps)
            # kp replicated along an innermost qi axis, once per chunk (the
            # content is qi-invariant). This makes both add operands 16-bit
            # with step-1 innermost dims, so the DVE tensor_tensor runs in
            # 2x packed mode. Built by doubling copies so the replication
            # itself runs in DVE packed modes too.
            kp_rep = proj_sb.tile([128, 2, KC, QG], BF16, tag="kp_rep")
            nc.vector.tensor_copy(kp_rep[:, :, :, 0:1], _bcast(kp_ps, 3, 1))
            w = 1
            while w < QG:
                nc.vector.tensor_copy(
                    kp_rep[:, :, :, w : 2 * w], kp_rep[:, :, :, 0:w]
                )
                w *= 2

            # ---- per-group: adds -> tanh -> score columns ----
            # Ramped group sizes: small first groups get ACT started early on
            # the first chunk; small last groups shrink the serial tail on the
            # last chunk.
            if U == 1:
                sizes = [8, 12, 18, 26, 32, 16, 8, 8]
            elif u == 0:
                sizes = [8, 12, 18, 26, 32, 32]
            elif u == U - 1:
                sizes = [32, 32, 32, 16, 8, 8]
            else:
                sizes = [32, 32, 32, 32]
            scT_ps = sc_ps_pool.tile([128, Tq], F32, tag="scT")
            pT_sb = pt_pool.tile([128, Tq], BF16, tag="pT")
            q0 = 0
            for qg in sizes:
                qs = slice(q0, q0 + qg)
                # qi-innermost layout: both TT operands are bf16 with step-1
                # innermost (kp_rep directly, qp with a step-0 k dim in the
                # middle), which enables the DVE 2x packed mode.
                ssum = ssum_pool.tile([128, 2, KC, QG], BF16, tag="ssum")
                nc.vector.tensor_tensor(
                    out=ssum[:, :, :, :qg],
                    in0=kp_rep[:, :, :, :qg],
                    in1=_bcast(qpT_sb[:, :, qs], 2, KC),
                    op=mybir.AluOpType.add,
                )
                feat = feat_pool.tile([128, 2, KC, QG], BF16, tag="feat")
                nc.scalar.activation(
                    feat[:, :, :, :qg],
                    ssum[:, :, :, :qg],
                    mybir.ActivationFunctionType.Tanh,
                )
                for qi in range(qg):
                    q = q0 + qi
                    for half in range(2):
                        nc.tensor.matmul(
                            scT_ps[:, q : q + 1],
                            feat[:, half, :, qi],
                            wv_sb[:, half : half + 1],
                            start=(half == 0),
                            stop=(half == 1),
                        )
                q0 += qg

            # ---- exp with mask/bound bias; partial AV with denominator ----
            nc.scalar.activation(
                pT_sb,
                scT_ps,
                mybir.ActivationFunctionType.Exp,
                bias=mb_sb[:, 0:1],
                scale=1.0,
            )
            av_ps = av_ps_pool.tile([Tq, D + 1], F32, tag="avo")
            nc.tensor.matmul(av_ps, pT_sb, v_sb, start=True, stop=True)
            out_sb = out_pool.tile([Tq, D + 1], F32, tag="out")
            nc.vector.tensor_copy(out_sb, av_ps)
            nc.sync.dma_start(out=out_dram[u], in_=out_sb)


def _build(U):
    nc = bacc.Bacc(
        "TRN2",
        target_bir_lowering=False,
        debug=False,
        enable_asserts=False,
        num_devices=N_CORES,
    )
    ins = {
        "wq": nc.dram_tensor("wq", [D, H], BF16, kind="ExternalInput").ap(),
        "wk": nc.dram_tensor("wk", [D, H], BF16, kind="ExternalInput").ap(),
        "wv": nc.dram_tensor("wv", [H], BF16, kind="ExternalInput").ap(),
        "qT_u": nc.dram_tensor("qT_u", [U, D, Tq], BF16, kind="ExternalInput").ap(),
        "kT_u": nc.dram_tensor("kT_u", [U, D, KC], BF16, kind="ExternalInput").ap(),
        "v_u": nc.dram_tensor("v_u", [U, KC, D + 1], BF16, kind="ExternalInput").ap(),
        "mb_u": nc.dram_tensor("mb_u", [U, KC, 1], F32, kind="ExternalInput").ap(),
    }
    out_dram = nc.dram_tensor("out_u", [U, Tq, D + 1], F32, kind="ExternalOutput").ap()
    with tile.TileContext(nc) as tc:
        _emit(nc, tc, ins, out_dram, U)
    nc.compile()
    return nc


_NC_CACHE = {}


def _get_nc(U):
    if U not in _NC_CACHE:
        _NC_CACHE[U] = _build(U)
    return _NC_CACHE[U]


def _plan_chunks(valid_lens):
    chunks = []
    for b in range(B):
        n = int(valid_lens[b])
        for kc in range(math.ceil(max(n, 0) / KC)):
            chunks.append((b, kc))
    U = max(1, math.ceil(len(chunks) / N_CORES))
    chunks += [None] * (N_CORES * U - len(chunks))
    return chunks, U


def run(queries, keys, values, valid_lens, Wq, Wk, wv, trace=False):
    """Run the SPMD kernel; returns (output, BassKernelResults)."""
    queries = np.asarray(queries, dtype=np.float32)
    keys = np.asarray(keys, dtype=np.float32)
    values = np.asarray(values, dtype=np.float32)
    valid_lens = np.asarray(valid_lens)
    Wq = np.ascontiguousarray(np.asarray(Wq, dtype=np.float32)).astype(ml_dtypes.bfloat16)
    Wk = np.ascontiguousarray(np.asarray(Wk, dtype=np.float32)).astype(ml_dtypes.bfloat16)
    wv_bf = np.asarray(wv, dtype=np.float32).astype(ml_dtypes.bfloat16)
    # scores are bounded by sum|wv| since |tanh| <= 1; M makes exp(s-M) safe
    # without any row max, so partial softmax sums combine by addition.
    M = float(np.abs(wv_bf.astype(np.float32)).sum()) + 1.0

    chunks, U = _plan_chunks(valid_lens)
    nc = _get_nc(U)

    qT = np.ascontiguousarray(queries.transpose(0, 2, 1)).astype(ml_dtypes.bfloat16)
    kT = np.ascontiguousarray(keys.transpose(0, 2, 1)).astype(ml_dtypes.bfloat16)
    ones = np.ones((KC, 1), dtype=np.float32)
    arange = np.arange(KC)

    in_maps = []
    for c in range(N_CORES):
        qT_u = np.zeros((U, D, Tq), ml_dtypes.bfloat16)
        kT_u = np.zeros((U, D, KC), ml_dtypes.bfloat16)
        v_u = np.zeros((U, KC, D + 1), ml_dtypes.bfloat16)
        mb_u = np.full((U, KC, 1), NEG_BIG - M, np.float32)
        for u in range(U):
            ch = chunks[c * U + u]
            if ch is None:
                continue
            b, kc = ch
            k0 = kc * KC
            qT_u[u] = qT[b]
            kT_u[u] = kT[b][:, k0 : k0 + KC]
            v_u[u] = np.concatenate([values[b][k0 : k0 + KC], ones], axis=1).astype(
                ml_dtypes.bfloat16
            )
            mb_u[u, :, 0] = (
                np.where(k0 + arange < int(valid_lens[b]), 0.0, NEG_BIG) - M
            ).astype(np.float32)
        in_maps.append(
            {
                "wq": Wq,
                "wk": Wk,
                "wv": wv_bf,
                "qT_u": qT_u,
                "kT_u": kT_u,
                "v_u": v_u,
                "mb_u": mb_u,
            }
        )

    res = bass_utils.run_bass_kernel_spmd(
        nc, in_maps, core_ids=list(range(N_CORES)), trace=trace
    )

    acc = np.zeros((B, Tq, D + 1), np.float64)
    for c in range(N_CORES):
        part = res.results[c]["out_u"]  # [U, Tq, D+1]
        for u in range(U):
            ch = chunks[c * U + u]
            if ch is None:
                continue
            acc[ch[0]] += part[u]
    out = np.zeros((B, Tq, D), np.float32)
    for b in range(B):
        if int(valid_lens[b]) > 0:
            out[b] = (acc[b, :, :D] / acc[b, :, D : D + 1]).astype(np.float32)
    return out, res


def kernel(queries, keys, values, valid_lens, Wq, Wk, wv):
    out, _ = run(queries, keys, values, valid_lens, Wq, Wk, wv, trace=False)
    return out


# revision 17
# speedup vs baseline: 1.0211x; 1.0211x over previous
"""Trainium2 Bass kernel for additive (Bahdanau) attention.

Reference computation (per batch b):
    qp = queries @ Wq                    # (Tq, H)
    kp = keys @ Wk                       # (Tk, H)
    scores[q,k] = sum_h wv[h] * tanh(qp[q,h] + kp[k,h])
    attn = softmax(scores masked to k < valid_lens[b])
    out = attn @ values                  # (Tq, D)

Shapes: B=8, Tq=128, Tk=512, D=256, H=256 (fp32).

Strategy (v2 — valid-length-balanced key-axis sharding):

The dominant cost is the (b, q, k, h) feature tensor, but keys with
k >= valid_lens[b] never influence the output, so only
sum_b ceil(len_b/128) 128-key chunks of work exist. The host enumerates
those chunks (b, kc), pads to a uniform U = ceil(n/8) per core with dummy
(fully-masked) chunks, and ships per-chunk inputs; every core runs the
same static program over U chunks.

Per chunk, on-core (feature axis H on partitions, two halves of 128):
  - qp/kp projections on TensorE from pre-transposed qT/kT slices;
  - qp+kp broadcast-add via DVE/GPSIMD tensor_tensor with step-0
    access-pattern broadcasts (qp broadcast along k, kp along q), the key
    columns split 88:40 between VectorE and GpSimd so both stay under the
    tanh pace;
  - tanh on ScalarE over [128, 2*32*128] tiles -> bf16 features;
  - wv contraction on TensorE with the feature tile as the stationary
    operand (lhsT [128h, 128k]) and the wv half [128,1] as the moving
    operand, accumulating transposed score columns scT[k, q] in PSUM;
  - exp(scT + bias) where bias = mask - M (M = sum|wv| bounds all scores,
    so no row max is needed and cross-chunk partials combine by plain
    summation); k is the partition axis so the mask is a per-partition
    bias on the one exp instruction;
  - attn-partial @ [values | 1] on TensorE: a ones-column appended to V
    accumulates the softmax denominator as column 256.
The host sums the per-chunk [128, 257] partials per batch and divides by
column 256 to produce the normalized output.
"""

import math
import numpy as np
import ml_dtypes
from contextlib import ExitStack

import concourse.bass as bass
import concourse.tile as tile
from concourse import bacc, mybir
from concourse import bass_utils

B, Tq, Tk, D, H = 8, 128, 512, 256, 256
N_CORES = 8
KC = 128          # keys per chunk
QG = 32           # queries per tanh group
NGRP = Tq // QG   # 4 groups per chunk
# Key columns of each add on DVE vs GpSimd. GpSimd's software tensor_tensor
# loop costs ~74ns per qi-row regardless of length, so it can only carry a
# small share before falling behind the tanh pace.
KS_DVE = 108
F32 = mybir.dt.float32
BF16 = mybir.dt.bfloat16
NEG_BIG = -1.0e9


def _bcast(ap_slice, axis_idx, count):
    """Insert a step-0 (broadcast) dim into an AP at free-axis position."""
    ap = list(ap_slice.ap)
    ap.insert(axis_idx, [0, count])
    return bass.AP(tensor=ap_slice.tensor, offset=ap_slice.offset, ap=ap)


def _emit(nc, tc, ins, out_dram, U):
    with ExitStack() as ctx:
        const = ctx.enter_context(tc.tile_pool(name="const", bufs=1))
        chunk_in = ctx.enter_context(tc.tile_pool(name="chunk_in", bufs=2))
        proj_sb = ctx.enter_context(tc.tile_pool(name="proj_sb", bufs=2))
        ssum_pool = ctx.enter_context(tc.tile_pool(name="ssum", bufs=2))
        feat_pool = ctx.enter_context(tc.tile_pool(name="feat", bufs=2))
        pt_pool = ctx.enter_context(tc.tile_pool(name="pt", bufs=2))
        out_pool = ctx.enter_context(tc.tile_pool(name="outs", bufs=2))
        proj_ps = ctx.enter_context(tc.tile_pool(name="proj_ps", bufs=2, space="PSUM"))
        sc_ps_pool = ctx.enter_context(tc.tile_pool(name="sc_ps", bufs=2, space="PSUM"))
        av_ps_pool = ctx.enter_context(tc.tile_pool(name="av_ps", bufs=2, space="PSUM"))

        # Dummy activation with no data dependencies: pulls the ACT table
        # load (~1.3us, tanh+exp share one set) off the first tanh's
        # critical path — it runs concurrently with the input DMAs.
        warm_sb = const.tile([1, 1], F32)
        nc.vector.memset(warm_sb, 0.0)
        nc.scalar.activation(warm_sb, warm_sb, mybir.ActivationFunctionType.Tanh)

        # Two HWDGE issue queues (sync + scalar). Chunk-0 critical path runs
        # kT/wk -> kp projection -> kp_rep chain, so those loads go first.
        wk_sb = const.tile([128, 2, H], BF16)
        wq_sb = const.tile([128, 2, H], BF16)
        wv_sb = const.tile([128, 2], BF16)

        for u in range(U):
            # ---- chunk input loads ----
            kT_sb = chunk_in.tile([128, 2, KC], BF16, tag="kT")
            nc.sync.dma_start(
                out=kT_sb, in_=ins["kT_u"][u].rearrange("(c p) k -> p c k", p=128)
            )
            qT_sb = chunk_in.tile([128, 2, Tq], BF16, tag="qT")
            nc.scalar.dma_start(
                out=qT_sb, in_=ins["qT_u"][u].rearrange("(c p) q -> p c q", p=128)
            )
            if u == 0:
                nc.sync.dma_start(
                    out=wk_sb, in_=ins["wk"].rearrange("(c p) h -> p c h", p=128)
                )
                nc.scalar.dma_start(
                    out=wq_sb, in_=ins["wq"].rearrange("(c p) h -> p c h", p=128)
                )
                nc.scalar.dma_start(
                    out=wv_sb, in_=ins["wv"].rearrange("(c p) -> p c", p=128)
                )
            v_sb = chunk_in.tile([128, D + 1], BF16, tag="v")
            nc.sync.dma_start(out=v_sb, in_=ins["v_u"][u])
            mb_sb = chunk_in.tile([128, 1], F32, tag="mb")
            nc.sync.dma_start(out=mb_sb, in_=ins["mb_u"][u])

            # ---- projections (kp first: it feeds the longer copy chain) ----
            qp_ps = proj_ps.tile([128, 2, Tq], F32, tag="qp_ps")
            kp_ps = proj_ps.tile([128, 2, KC], F32, tag="kp_ps")
            for half in range(2):
                hs = slice(half * 128, (half + 1) * 128)
                for dc in range(2):
                    nc.tensor.matmul(
                        kp_ps[:, half, :],
                        wk_sb[:, dc, hs],
                        kT_sb[:, dc, :],
                        start=(dc == 0),
                        stop=(dc == 1),
                    )
            for half in range(2):
                hs = slice(half * 128, (half + 1) * 128)
                for dc in range(2):
                    nc.tensor.matmul(
                        qp_ps[:, half, :],
                        wq_sb[:, dc, hs],
                        qT_sb[:, dc, :],
                        start=(dc == 0),
                        stop=(dc == 1),
                    )
            # On chunk 0 the scalar engine is still idle, so routing the
            # qpT PSUM->SBUF copy there lets the DVE kp_rep chain below run
            # concurrently and shortens the preamble.
            qpT_sb = proj_sb.tile([128, 2, Tq], BF16, tag="qpT")
            if u == 0:
                nc.scalar.copy(qpT_sb, qp_ps)
            else:
                nc.vector.tensor_copy(qpT_sb, qp_ps)
            # kp replicated along an innermost qi axis, once per chunk (the
            # content is qi-invariant). This makes both add operands 16-bit
            # with step-1 innermost dims, so the DVE tensor_tensor runs in
            # 2x packed mode. Built by doubling copies so the replication
            # itself runs in DVE packed modes too.
            kp_rep = proj_sb.tile([128, 2, KC, QG], BF16, tag="kp_rep")
            # seed at width 2 directly: 1-element rows pay a per-row bubble
            nc.vector.tensor_copy(kp_rep[:, :, :, 0:2], _bcast(kp_ps, 3, 2))
            w = 2
            while w < QG:
                nc.vector.tensor_copy(
                    kp_rep[:, :, :, w : 2 * w], kp_rep[:, :, :, 0:w]
                )
                w *= 2

            # ---- per-group: adds -> tanh -> score columns ----
            # Ramped group sizes: small first groups get ACT started early on
            # the first chunk; small last groups shrink the serial tail on the
            # last chunk.
            if U == 1:
                sizes = [8, 12, 18, 26, 32, 16, 8, 8]
            elif u == 0:
                sizes = [8, 12, 18, 26, 32, 32]
            elif u == U - 1:
                sizes = [32, 32, 32, 16, 8, 8]
            else:
                sizes = [32, 32, 32, 32]
            scT_ps = sc_ps_pool.tile([128, Tq], F32, tag="scT")
            pT_sb = pt_pool.tile([128, Tq], BF16, tag="pT")
            q0 = 0
            for qg in sizes:
                qs = slice(q0, q0 + qg)
                # qi-innermost layout: both TT operands are bf16 with step-1
                # innermost (kp_rep directly, qp with a step-0 k dim in the
                # middle), which enables the DVE 2x packed mode.
                ssum = ssum_pool.tile([128, 2, KC, QG], BF16, tag="ssum")
                nc.vector.tensor_tensor(
                    out=ssum[:, :, :, :qg],
                    in0=kp_rep[:, :, :, :qg],
                    in1=_bcast(qpT_sb[:, :, qs], 2, KC),
                    op=mybir.AluOpType.add,
                )
                feat = feat_pool.tile([128, 2, KC, QG], BF16, tag="feat")
                nc.scalar.activation(
                    feat[:, :, :, :qg],
                    ssum[:, :, :, :qg],
                    mybir.ActivationFunctionType.Tanh,
                )
                for qi in range(qg):
                    q = q0 + qi
                    for half in range(2):
                        nc.tensor.matmul(
                            scT_ps[:, q : q + 1],
                            feat[:, half, :, qi],
                            wv_sb[:, half : half + 1],
                            start=(half == 0),
                            stop=(half == 1),
                        )
                q0 += qg

            # ---- exp with mask/bound bias; partial AV with denominator ----
            nc.scalar.activation(
                pT_sb,
                scT_ps,
                mybir.ActivationFunctionType.Exp,
                bias=mb_sb[:, 0:1],
                scale=1.0,
            )
            av_ps = av_ps_pool.tile([Tq, D + 1], F32, tag="avo")
            nc.tensor.matmul(av_ps, pT_sb, v_sb, start=True, stop=True)
            out_sb = out_pool.tile([Tq, D + 1], F32, tag="out")
            nc.vector.tensor_copy(out_sb, av_ps)
            nc.sync.dma_start(out=out_dram[u], in_=out_sb)


def _build(U):
    nc = bacc.Bacc(
        "TRN2",
        target_bir_lowering=False,
        debug=False,
        enable_asserts=False,
        num_devices=N_CORES,
    )
    ins = {
        "wq": nc.dram_tensor("wq", [D, H], BF16, kind="ExternalInput").ap(),
        "wk": nc.dram_tensor("wk", [D, H], BF16, kind="ExternalInput").ap(),
        "wv": nc.dram_tensor("wv", [H], BF16, kind="ExternalInput").ap(),
        "qT_u": nc.dram_tensor("qT_u", [U, D, Tq], BF16, kind="ExternalInput").ap(),
        "kT_u": nc.dram_tensor("kT_u", [U, D, KC], BF16, kind="ExternalInput").ap(),
        "v_u": nc.dram_tensor("v_u", [U, KC, D + 1], BF16, kind="ExternalInput").ap(),
        "mb_u": nc.dram_tensor("mb_u", [U, KC, 1], F32, kind="ExternalInput").ap(),
    }
    out_dram = nc.dram_tensor("out_u", [U, Tq, D + 1], F32, kind="ExternalOutput").ap()
    with tile.TileContext(nc) as tc:
        _emit(nc, tc, ins, out_dram, U)
    nc.compile()
    return nc


_NC_CACHE = {}


def _get_nc(U):
    if U not in _NC_CACHE:
        _NC_CACHE[U] = _build(U)
    return _NC_CACHE[U]


def _plan_chunks(valid_lens):
    chunks = []
    for b in range(B):
        n = int(valid_lens[b])
        for kc in range(math.ceil(max(n, 0) / KC)):
            chunks.append((b, kc))
    U = max(1, math.ceil(len(chunks) / N_CORES))
    chunks += [None] * (N_CORES * U - len(chunks))
    return chunks, U


def run(queries, keys, values, valid_lens, Wq, Wk, wv, trace=False):
    """Run the SPMD kernel; returns (output, BassKernelResults)."""
    queries = np.asarray(queries, dtype=np.float32)
    keys = np.asarray(keys, dtype=np.float32)
    values = np.asarray(values, dtype=np.float32)
    valid_lens = np.asarray(valid_lens)
    Wq = np.ascontiguousarray(np.asarray(Wq, dtype=np.float32)).astype(ml_dtypes.bfloat16)
    Wk = np.ascontiguousarray(np.asarray(Wk, dtype=np.float32)).astype(ml_dtypes.bfloat16)
    wv_bf = np.asarray(wv, dtype=np.float32).astype(ml_dtypes.bfloat16)
    # scores are bounded by sum|wv| since |tanh| <= 1; M makes exp(s-M) safe
    # without any row max, so partial softmax sums combine by addition.
    M = float(np.abs(wv_bf.astype(np.float32)).sum()) + 1.0

    chunks, U = _plan_chunks(valid_lens)
    nc = _get_nc(U)

    qT = np.ascontiguousarray(queries.transpose(0, 2, 1)).astype(ml_dtypes.bfloat16)
    kT = np.ascontiguousarray(keys.transpose(0, 2, 1)).astype(ml_dtypes.bfloat16)
    ones = np.ones((KC, 1), dtype=np.float32)
    arange = np.arange(KC)

    in_maps = []
    for c in range(N_CORES):
        qT_u = np.zeros((U, D, Tq), ml_dtypes.bfloat16)
        kT_u = np.zeros((U, D, KC), ml_dtypes.bfloat16)
        v_u = np.zeros((U, KC, D + 1), ml_dtypes.bfloat16)
        mb_u = np.full((U, KC, 1), NEG_BIG - M, np.float32)
        for u in range(U):
            ch = chunks[c * U + u]
            if ch is None:
                continue
            b, kc = ch
            k0 = kc * KC
            qT_u[u] = qT[b]
            kT_u[u] = kT[b][:, k0 : k0 + KC]
            v_u[u] = np.concatenate([values[b][k0 : k0 + KC], ones], axis=1).astype(
                ml_dtypes.bfloat16
            )
            mb_u[u, :, 0] = (
                np.where(k0 + arange < int(valid_lens[b]), 0.0, NEG_BIG) - M
            ).astype(np.float32)
        in_maps.append(
            {
                "wq": Wq,
                "wk": Wk,
                "wv": wv_bf,
                "qT_u": qT_u,
                "kT_u": kT_u,
                "v_u": v_u,
                "mb_u": mb_u,
            }
        )

    res = bass_utils.run_bass_kernel_spmd(
        nc, in_maps, core_ids=list(range(N_CORES)), trace=trace
    )

    acc = np.zeros((B, Tq, D + 1), np.float64)
    for c in range(N_CORES):
        part = res.results[c]["out_u"]  # [U, Tq, D+1]
        for u in range(U):
            ch = chunks[c * U + u]
            if ch is None:
                continue
            acc[ch[0]] += part[u]
    out = np.zeros((B, Tq, D), np.float32)
    for b in range(B):
        if int(valid_lens[b]) > 0:
            out[b] = (acc[b, :, :D] / acc[b, :, D : D + 1]).astype(np.float32)
    return out, res


def kernel(queries, keys, values, valid_lens, Wq, Wk, wv):
    out, _ = run(queries, keys, values, valid_lens, Wq, Wk, wv, trace=False)
    return out
